# revision 2
# baseline (speedup 1.0000x reference)
"""2D-DCT (DCT-II, orthonormal) spatial transform on Trainium2, 8 NeuronCores.

Full input x [16,256,128,128] f32 -> out[b,c,k,v] = sum_hw Wy[k,h] Wx[v,w] x[b,c,h,w]
with Wy = Wx = 128-point orthonormal DCT-II matrix W.

Strategy (data-parallel, batch*channel sharded 4096 -> 512 images/core):
per image X: out = W @ X @ W.T, computed with two PE matmuls and zero
explicit transposes -- matmul(out, lhsT, rhs) = lhsT.T @ rhs transposes the
stationary operand for free:
  mm1: lhsT=X_i,   rhs=W.T  -> Z^T = (W@X)^T   (PSUM, fp32 accum)
  mm2: lhsT=Z^T_i, rhs=W.T  -> out             (PSUM, fp32 accum)

All device I/O and matmul operands are bf16 (fp32 accumulation in PSUM):
- halves HBM traffic vs f32 (64KB/image total) -> DMA roofline ~183 ns/img
  at the ~358 GB/s per-core HBM limit, vs ~366 ns/img for f32 I/O.
- bf16 matmuls run 1 cycle/row (128 cyc/MM) and weight loads get FWL (2
  elems/cycle), so PE stays under the DMA floor (~160 ns/img incl LDW).
- error is ~3e-3 scale-relative absmax (inputs/weights rounded to 8-bit
  mantissa; orthonormal transform preserves the relative error), well
  inside the 2e-2 tolerance.

Host packs images in groups of 8 as [n_grp, 128(h), 8(img), 128(w)] so
every DMA moves 2KB contiguous per partition (vs 512B chunks when images
are kept separate) on both the load and store side. PSUM->SBUF cast
copies are batched 4 images per instruction, stage 1 on ACT, stage 2 on
DVE, keeping each vector engine at ~100-150 ns/img.
"""

import sys

for _p in ("/opt/trn_rl_repo", "/root/.axon_site/_ro/trn_rl_repo"):
    if _p not in sys.path:
        sys.path.insert(0, _p)

import numpy as np
import ml_dtypes

BF16 = ml_dtypes.bfloat16

N_CORES = 8
B, C, H, W = 16, 256, 128, 128
PER_CORE = B * C // N_CORES  # 512 images per core
GROUP = 8                    # images per DMA group


def _dct_matrix(n: int) -> np.ndarray:
    v = np.arange(n, dtype=np.float64)[:, None]
    j = np.arange(n, dtype=np.float64)[None, :]
    f = np.cos(np.pi * (0.5 + j) * v / n) / np.sqrt(n)
    f *= np.where(v != 0, np.sqrt(2.0), 1.0)
    return f.astype(np.float32)


def _build_program(n_img: int, group: int = GROUP, xg_bufs: int = 4, og_bufs: int = 4,
                   p1_bufs: int = 3, p2_bufs: int = 3, zt_bufs: int = 3):
    import concourse.bacc as bacc_mod
    import concourse.mybir as mybir
    from concourse.tile import TileContext

    F32 = mybir.dt.float32
    BF = mybir.dt.bfloat16
    n_grp = n_img // group

    nc = bacc_mod.Bacc()
    x = nc.declare_dram_parameter("x", [n_grp, 128, group, 128], BF, isOutput=False)
    wt_p = nc.declare_dram_parameter("wt", [128, 128], BF, isOutput=False)
    out = nc.declare_dram_parameter("out", [n_grp, 128, group, 128], BF, isOutput=True)

    with TileContext(nc) as tc:
        with tc.tile_pool(name="consts", bufs=1) as cpool, \
             tc.tile_pool(name="xin", bufs=xg_bufs) as xpool, \
             tc.tile_pool(name="mid", bufs=zt_bufs) as zpool, \
             tc.tile_pool(name="oput", bufs=og_bufs) as opool, \
             tc.tile_pool(name="ps", bufs=1, space="PSUM") as pspool:
            wt = cpool.tile([128, 128], BF)
            nc.gpsimd.dma_start(out=wt, in_=wt_p[:])

            # PE warm-up dummy: ensure no later (self-loading) matmul needs
            # more than one sync wait -- the S3_LW struct can carry only one.
            pdum = pspool.tile([128, 128], F32, tag="pdum", bufs=1)
            nc.tensor.matmul(pdum, lhsT=wt, rhs=wt, start=True, stop=True)

            for g in range(n_grp):
                xg = xpool.tile([128, group, 128], BF, tag="xg")
                nc.sync.dma_start(out=xg, in_=x[g])
                og = opool.tile([128, group, 128], BF, tag="og")
                for q in range(group // 4):
                    p1 = pspool.tile([128, 4, 128], F32, tag="p1", bufs=p1_bufs)
                    for i in range(4):
                        nc.tensor.matmul(p1[:, i, :], lhsT=xg[:, q * 4 + i, :],
                                         rhs=wt, start=True, stop=True)
                    zt = zpool.tile([128, 4, 128], BF, tag="zt")
                    nc.scalar.copy(out=zt, in_=p1)  # batched cast copy (ACT)
                    p2 = pspool.tile([128, 4, 128], F32, tag="p2", bufs=p2_bufs)
                    for i in range(4):
                        nc.tensor.matmul(p2[:, i, :], lhsT=zt[:, i, :],
                                         rhs=wt, start=True, stop=True)
                    nc.vector.tensor_copy(out=og[:, q * 4:(q + 1) * 4, :], in_=p2)
                nc.sync.dma_start(out=out[g], in_=og)
    nc.finalize()
    return nc


_CACHE = {}


def _pack(flat_bf: np.ndarray) -> np.ndarray:
    """[n,128,128] bf16 -> [n/GROUP, 128, GROUP, 128] bf16 (row-interleaved)."""
    n = flat_bf.shape[0]
    return np.ascontiguousarray(
        flat_bf.reshape(n // GROUP, GROUP, 128, 128).transpose(0, 2, 1, 3))


def _unpack(packed_bf: np.ndarray) -> np.ndarray:
    """[n/GROUP, 128, GROUP, 128] bf16 -> [n,128,128] f32."""
    n_grp = packed_bf.shape[0]
    return packed_bf.transpose(0, 2, 1, 3).astype(np.float32).reshape(
        n_grp * GROUP, 128, 128)


def kernel(x: np.ndarray) -> np.ndarray:
    from concourse.bass_utils import run_bass_kernel_spmd

    assert x.shape == (B, C, H, W), x.shape
    xb = np.ascontiguousarray(x, dtype=np.float32).reshape(B * C, H, W).astype(BF16)
    xp = _pack(xb)  # [512, 128, 8, 128] over all cores

    if "nc" not in _CACHE:
        _CACHE["nc"] = _build_program(PER_CORE)
    nc = _CACHE["nc"]

    wt = _dct_matrix(128).T.copy().astype(BF16)  # WT[h,k] = W[k,h]

    gpc = PER_CORE // GROUP  # groups per core
    in_maps = [
        {"x": xp[c * gpc:(c + 1) * gpc], "wt": wt}
        for c in range(N_CORES)
    ]
    res = run_bass_kernel_spmd(nc, in_maps, list(range(N_CORES)))
    packed = np.concatenate([r["out"] for r in res.results], axis=0)
    return _unpack(packed).reshape(B, C, H, W)


if __name__ == "__main__":
    rng = np.random.default_rng(0)
    xs = rng.standard_normal((B, C, H, W), dtype=np.float32)
    o = kernel(xs)
    print("kernel output", o.shape, o.dtype)


# revision 8
# speedup vs baseline: 1.7054x; 1.7054x over previous
"""2D-DCT (DCT-II, orthonormal) spatial transform on Trainium2, 8 NeuronCores.

Full input x [16,256,128,128] f32 -> out[b,c,k,v] = sum_hw Wy[k,h] Wx[v,w] x[b,c,h,w]
with Wy = Wx = 128-point orthonormal DCT-II matrix W.

Strategy (data-parallel, batch*channel sharded 4096 -> 512 images/core):
per image X: out = W @ X @ W.T via two PE matmuls, no explicit transposes --
matmul(out, lhsT, rhs) = lhsT.T @ rhs transposes the stationary operand free:
  mm1 (per image):     lhsT=X_i, rhs=W.T          -> Z^T = (W@X)^T    (PSUM)
  mm2 (4-image batch): lhsT=W.T, rhs=[Z^T x4]     -> [out^T x4]       (PSUM)
mm2 keeps the tiny DCT matrix stationary across the whole kernel (no
weight reloads, N=512 moving free dim), and the transposed output layout
is absorbed by the host-side unpack transpose.

All device I/O and matmul operands are bf16 (fp32 accumulation in PSUM):
- halves HBM traffic vs f32 (64KB/image total) -> DMA roofline ~183-197
  ns/img at the ~358 GB/s per-core HBM limit.
- bf16 matmuls run 1 cycle/row and weight loads get FWL, so PE stays
  under the DMA floor (~135 ns/img incl per-image LDWEIGHTS for mm1).
- error ~4e-3 scale-relative absmax (8-bit mantissa inputs/weights;
  orthonormal transform preserves relative error), inside the 2e-2 gate.

Host packs images in groups of 16 as [n_grp, 128(h), 16(img), 128(w)] so
every DMA moves 4KB contiguous per partition on both load and store
(measured DMA-only: 2KB chunks 240 ns/img, 4KB chunks 225 ns/img).
PSUM->SBUF cast copies are batched 4 images per instruction: stage 1 on
ACT, stage 2 on DVE.

Measured by repetition-slope on silicon (8 cores concurrent, device-
resident buffers, For_i(R)-looped body, R=500 vs 8000, walls stable to
+-0.2ms): 241-247 ns/img end-to-end.
"""

import sys

for _p in ("/opt/trn_rl_repo", "/root/.axon_site/_ro/trn_rl_repo"):
    if _p not in sys.path:
        sys.path.insert(0, _p)

import numpy as np
import ml_dtypes

BF16 = ml_dtypes.bfloat16

N_CORES = 8
B, C, H, W = 16, 256, 128, 128
PER_CORE = B * C // N_CORES  # 512 images per core
GROUP = 16                   # images per DMA group (4KB/partition descriptors)


def _dct_matrix(n: int) -> np.ndarray:
    v = np.arange(n, dtype=np.float64)[:, None]
    j = np.arange(n, dtype=np.float64)[None, :]
    f = np.cos(np.pi * (0.5 + j) * v / n) / np.sqrt(n)
    f *= np.where(v != 0, np.sqrt(2.0), 1.0)
    return f.astype(np.float32)


def _build_program(n_img: int, group: int = GROUP, xg_bufs: int = 4, og_bufs: int = 4,
                   p1_bufs: int = 3, p2_bufs: int = 3, zt_bufs: int = 3,
                   reps: int | None = None, dma_only: bool = False,
                   no_dma: bool = False):
    """Build the per-core program. reps!=None wraps the image loop in a
    hardware For_i loop (for repetition-slope timing). dma_only / no_dma
    build crippled variants for roofline measurement (wrong results)."""
    import concourse.bacc as bacc_mod
    import concourse.mybir as mybir
    from concourse.tile import TileContext
    from contextlib import nullcontext

    F32 = mybir.dt.float32
    BF = mybir.dt.bfloat16
    n_grp = n_img // group

    nc = bacc_mod.Bacc()
    x = nc.declare_dram_parameter("x", [n_grp, 128, group, 128], BF, isOutput=False)
    wt_p = nc.declare_dram_parameter("wt", [128, 128], BF, isOutput=False)
    out = nc.declare_dram_parameter("out", [n_grp, 128, group, 128], BF, isOutput=True)

    with TileContext(nc) as tc:
        with tc.tile_pool(name="consts", bufs=1) as cpool, \
             tc.tile_pool(name="xin", bufs=xg_bufs) as xpool, \
             tc.tile_pool(name="mid", bufs=zt_bufs) as zpool, \
             tc.tile_pool(name="oput", bufs=og_bufs) as opool, \
             tc.tile_pool(name="ps", bufs=1, space="PSUM") as pspool:
            wt = cpool.tile([128, 128], BF)
            nc.gpsimd.dma_start(out=wt, in_=wt_p[:])

            # PE warm-up dummy: ensure no later (self-loading) matmul needs
            # more than one sync wait -- the S3_LW struct can carry only one.
            pdum = pspool.tile([128, 128], F32, tag="pdum", bufs=1)
            nc.tensor.matmul(pdum, lhsT=wt, rhs=wt, start=True, stop=True)

            if no_dma:
                # resident input tiles, loaded once outside the timing loop
                xres = [cpool.tile([128, group, 128], BF, name=f"xres{i}")
                        for i in range(2)]
                for t in xres:
                    nc.sync.dma_start(out=t, in_=x[0])

            loop_cm = tc.For_i(0, reps) if reps is not None else nullcontext()
            with loop_cm:
                for g in range(n_grp):
                    if no_dma:
                        xg = xres[g % 2]
                    else:
                        xg = xpool.tile([128, group, 128], BF, tag="xg")
                        nc.sync.dma_start(out=xg, in_=x[g])
                    if dma_only:
                        # pure HBM traffic: bounce the input tile back out
                        nc.sync.dma_start(out=out[g], in_=xg)
                        continue
                    og = opool.tile([128, group, 128], BF, tag="og")
                    for q in range(group // 4):
                        p1 = pspool.tile([128, 4, 128], F32, tag="p1",
                                         bufs=p1_bufs)
                        for i in range(4):
                            nc.tensor.matmul(p1[:, i, :],
                                             lhsT=xg[:, q * 4 + i, :],
                                             rhs=wt, start=True, stop=True)
                        zt = zpool.tile([128, 4, 128], BF, tag="zt")
                        nc.scalar.copy(out=zt, in_=p1)  # ACT cast copy
                        p2 = pspool.tile([128, 4, 128], F32, tag="p2",
                                         bufs=p2_bufs)
                        # one matmul for 4 images: out^T = W @ Z^T, the
                        # DCT matrix stays stationary, N=512 moving
                        nc.tensor.matmul(p2, lhsT=wt, rhs=zt,
                                         start=True, stop=True)
                        nc.vector.tensor_copy(out=og[:, q * 4:(q + 1) * 4, :],
                                              in_=p2)
                    if not no_dma:
                        nc.sync.dma_start(out=out[g], in_=og)
    nc.finalize()
    return nc


_CACHE = {}


def _pack(flat_bf: np.ndarray, group: int = GROUP) -> np.ndarray:
    """[n,128,128] bf16 -> [n/group, 128(h), group, 128(w)] bf16."""
    n = flat_bf.shape[0]
    return np.ascontiguousarray(
        flat_bf.reshape(n // group, group, 128, 128).transpose(0, 2, 1, 3))


def _unpack(packed_bf: np.ndarray) -> np.ndarray:
    """[n/group, 128(v), group, 128(k)] bf16 (transposed images) -> [n,128,128] f32."""
    n_grp, _, group, _ = packed_bf.shape
    return packed_bf.transpose(0, 2, 3, 1).astype(np.float32).reshape(
        n_grp * group, 128, 128)


def kernel(x: np.ndarray) -> np.ndarray:
    from concourse.bass_utils import run_bass_kernel_spmd

    assert x.shape == (B, C, H, W), x.shape
    xb = np.ascontiguousarray(x, dtype=np.float32).reshape(B * C, H, W).astype(BF16)
    xp = _pack(xb)  # [512, 128, 8, 128] over all cores

    if "nc" not in _CACHE:
        _CACHE["nc"] = _build_program(PER_CORE)
    nc = _CACHE["nc"]

    wt = _dct_matrix(128).T.copy().astype(BF16)  # WT[h,k] = W[k,h]

    gpc = PER_CORE // GROUP  # groups per core
    in_maps = [
        {"x": xp[c * gpc:(c + 1) * gpc], "wt": wt}
        for c in range(N_CORES)
    ]
    res = run_bass_kernel_spmd(nc, in_maps, list(range(N_CORES)))
    packed = np.concatenate([r["out"] for r in res.results], axis=0)
    return _unpack(packed).reshape(B, C, H, W)


if __name__ == "__main__":
    rng = np.random.default_rng(0)
    xs = rng.standard_normal((B, C, H, W), dtype=np.float32)
    o = kernel(xs)
    print("kernel output", o.shape, o.dtype)
